# revision 34
# baseline (speedup 1.0000x reference)
"""Distributed Trainium2 kernel for nn_AlgebraicLinear (8, 4096, 256) x (256, 256) linear.

out[b, s, o] = sum_i x[b, s, i] * weight[o, i] + bias[o]

Sharding: pure data-parallel — batch dim (8) maps 1:1 onto the 8 NeuronCores.
Per core the GEMM is M=4096 tokens, K=256, N=256.

Layout: the host passes x[c].T (256, 4096) so the contraction axis i lands on
SBUF partitions with contiguous DMAs (no on-chip transpose). The device
computes out.T tiles (psum [o:128, s:512]) with float32r (FP22) matmuls; bias
is added during the PSUM->SBUF eviction (split across VectorE and ScalarE; it
is a per-partition scalar in this orientation). The host transposes the
returned out.T back. The weight W.T and bias are packed into one (128, 514)
host array so a single DMA loads all constants.

Raw bacc (no TileContext): hand-placed semaphores avoid Tile's multi-usec
end-of-kernel semaphore-reset butterfly; the Block exit barrier is skipped
(PE's final out_sem waits guarantee output completeness). Engine plan:
  Sync ring : ALL dma_starts — 5 inputs then 8 outputs; the single-ring FIFO
              yields a pure-read phase then a pure-write phase (HBM strongly
              prefers unmixed traffic)
  Tensor    : 32 matmuls (16 psum groups of K=2), 8 PSUM banks round-robin,
              then the final out_sem receipt waits (Tensor is last in the
              runtime's end-barrier chain, hiding the HBM write receipt)
  Vector    : evicts sh=0 half of each output block (tensor_scalar_add bias)
  Scalar    : evicts sh=1 half (activation Identity + per-partition bias)
"""

import numpy as np

B, S, I, O = 8, 4096, 256, 256
P = 128
SBLK = 1024
NS = S // SBLK    # 4 x-blocks
NH = SBLK // 512  # 2 psum halves per block
KT = I // P       # 2
OT = O // P       # 2
NB = NS * OT      # 8 output blocks
NG = NB * NH      # 16 psum groups
WB_COLS = KT * O + OT  # 514: [k*256+o] weights, then 2 bias cols
N_CORES = 8

_CACHE = {}


def _build():
    if "nc" in _CACHE:
        return _CACHE["nc"]

    import concourse.bass as bass  # noqa: F401
    import concourse.mybir as mybir
    from concourse import bacc
    from contextlib import ExitStack, contextmanager

    class _NoBarrierBlock(bass.BassBlock):
        """BassBlock whose exit skips the all-engine drain+barrier.

        Output completeness is guaranteed by the PE stream's final
        out_sem waits (each fires on DMA write receipt), so the ~7us
        drain/barrier teardown is pure measured-time overhead here.
        """

        def __exit__(self, exc_type, exc_val, exc_tb):
            if exc_type is None:
                for engine, last_body in self.last_body.items():
                    with self.bass.body(
                        last_body, parent=self.bass.cur_bb,
                        allow_existing_parent=True,
                    ):
                        engine.br(self.end_bb)
                self.bass.switch_bb(self.end_bb)

    @contextmanager
    def _no_barrier_block(nc):
        assert nc.cur_block is None
        with _NoBarrierBlock(nc, f"block_{nc.next_id()}") as blk:
            nc.cur_block = blk
            yield blk
        nc.cur_block = None

    f32 = mybir.dt.float32
    f32r = mybir.dt.float32r
    Act = mybir.ActivationFunctionType

    nc = bacc.Bacc("TRN2", target_bir_lowering=False, debug=False,
                   num_devices=N_CORES)

    xT_ext = nc.dram_tensor("xT", [I, S], f32r, kind="ExternalInput")
    wb_ext = nc.dram_tensor("wb", [P, WB_COLS], f32r, kind="ExternalInput")
    out_ext = nc.dram_tensor("out", [O, S], f32, kind="ExternalOutput")

    xT_d = xT_ext.ap().rearrange("(k p) s -> p k s", p=P)      # [128, 2, 4096]
    out_d = out_ext.ap().rearrange("(t p) s -> t p s", p=P)    # [2, 128, 4096]

    with ExitStack() as ctx:
        wb_sb = ctx.enter_context(nc.sbuf_tensor("wb_sb", [P, WB_COLS], f32r))
        # x chunks of 2048 cols: 2 MiB per dma_start, 8 KiB descriptors.
        CH = [2048, 2048]
        CH_OFF = [0, 2048]
        # col-segment (512-wide) index -> (chunk idx, col offset within chunk)
        SEG_CHUNK = [0, 0, 0, 0, 1, 1, 1, 1]
        SEG_OFF = [0, 512, 1024, 1536, 0, 512, 1024, 1536]
        x_sb = [ctx.enter_context(nc.sbuf_tensor(f"x_sb{i}", [P, KT, CH[i]], f32r))
                for i in range(len(CH))]
        o_sb = [ctx.enter_context(nc.sbuf_tensor(f"o_sb{i}", [P, SBLK], f32))
                for i in range(NB)]
        ps = [ctx.enter_context(nc.psum_tensor(f"ps{i}", [P, 512], f32))
              for i in range(8)]

        wb_sem = ctx.enter_context(nc.semaphore("wb_sem"))
        x_sem = [ctx.enter_context(nc.semaphore(f"x_sem{i}"))
                 for i in range(len(CH))]
        mm_sem = ctx.enter_context(nc.semaphore("mm_sem"))
        dve_sem = ctx.enter_context(nc.semaphore("dve_sem"))
        act_sem = ctx.enter_context(nc.semaphore("act_sem"))
        out_sem = [ctx.enter_context(nc.semaphore(f"out_sem{i}"))
                   for i in range(NB)]

        block = ctx.enter_context(_no_barrier_block(nc))

        def w_ap(k, ot):
            return wb_sb[:, k * O + ot * P:k * O + (ot + 1) * P]

        def bias_ap(ot):
            return wb_sb[:, KT * O + ot:KT * O + ot + 1].bitcast(f32)

        @block.sync
        def _(sp):
            # All DMA rides the single Sync HWDGE ring: the ring FIFO yields
            # a pure-read phase then a pure-write phase (HBM strongly
            # prefers unmixed traffic), with no extra semaphores.
            sp.dma_start(out=wb_sb[:], in_=wb_ext.ap()).then_inc(wb_sem, 16)
            for c in range(len(CH)):
                s0 = CH_OFF[c]
                sp.dma_start(
                    out=x_sb[c][:], in_=xT_d[:, :, s0:s0 + CH[c]]
                ).then_inc(x_sem[c], 16)
            for ob in range(NB):
                sb, ot = ob // 2, ob % 2
                sp.wait_ge(dve_sem, ob + 1)
                sp.wait_ge(act_sem, ob + 1)
                sp.dma_start(
                    out=out_d[ot][:, sb * SBLK:(sb + 1) * SBLK],
                    in_=o_sb[ob][:],
                ).then_inc(out_sem[ob], 16)

        @block.tensor
        def _(pe):
            waited_chunks = set()
            for g in range(NG):
                sb, ot, sh = g // 4, (g // 2) % 2, g % 2
                seg = sb * 2 + sh
                c, coff = SEG_CHUNK[seg], SEG_OFF[seg]
                if g == 0:
                    pe.wait_ge(wb_sem, 16)
                if c not in waited_chunks:
                    waited_chunks.add(c)
                    pe.wait_ge(x_sem[c], 16)
                if g == 8:
                    # Banks 0-7 are all free once every first-half group
                    # (0..7) has been evicted: two waits cover all eight
                    # bank-reuse hazards.
                    pe.wait_ge(dve_sem, 4)
                    pe.wait_ge(act_sem, 4)
                bank = ps[g % 8]
                for k in range(KT):
                    mm = nc.tensor.matmul(
                        bank[:],
                        lhsT=w_ap(k, ot),
                        rhs=x_sb[c][:, k, coff:coff + 512],
                        start=(k == 0),
                        stop=(k == KT - 1),
                    )
                mm.then_inc(mm_sem)
            # Kernel completion: every output byte landed in DRAM. These
            # waits live on PE because the runtime's final barrier chain
            # visits Tensor last — the HBM write receipt (~2-3us) then
            # overlaps the other engines' barrier hops instead of
            # preceding them.
            for ob in range(NB):
                pe.wait_ge(out_sem[ob], 16)

        @block.vector
        def _(dve):
            dve.wait_ge(wb_sem, 16)
            for ob in range(NB):
                g = 2 * ob
                ot = ob % 2
                dve.wait_ge(mm_sem, g + 1)
                nc.vector.tensor_scalar_add(
                    o_sb[ob][:, 0:512], ps[g % 8][:], bias_ap(ot)
                ).then_inc(dve_sem)

        @block.scalar
        def _(act):
            for ob in range(NB):
                g = 2 * ob + 1
                sb, ot = ob // 2, ob % 2
                act.wait_ge(mm_sem, g + 1)
                nc.scalar.activation(
                    o_sb[ob][:, 512:1024], ps[g % 8][:], Act.Identity,
                    bias=bias_ap(ot),
                ).then_inc(act_sem)

    # Strip the Bass-init preamble (4 unused const-tile memsets + the
    # all-engine barrier) from the head of main: every activation here uses
    # AP bias + immediate scale, so the const tiles have no readers, and the
    # data semaphores fully order the real work. Saves ~0.6us at exec start.
    for bb in nc.main_func.blocks:
        if bb.name == "main":
            drop = []
            for inst in bb.instructions:
                tn = type(inst).__name__
                if tn == "InstMemset" and inst.name in (
                        "I-34", "I-35", "I-36", "I-37"):
                    drop.append(inst)
                elif tn == "InstDrain" or tn == "InstEventSemaphore":
                    drop.append(inst)
                elif tn == "InstUnconditionalBranch":
                    break
            for inst in drop:
                bb.instructions.remove(inst)
                nc.inst_map.pop(inst.name, None)
            break

    nc.compile()
    _CACHE["nc"] = nc
    return nc


def _run(in_maps, trace=False, trace_kwargs=None):
    from concourse.bass_utils import run_bass_kernel_spmd

    nc = _build()
    return run_bass_kernel_spmd(
        nc, in_maps, core_ids=list(range(N_CORES)),
        trace=trace, **(trace_kwargs or {}),
    )


def _make_in_maps(x, weight, bias):
    x = np.asarray(x, dtype=np.float32)
    weight = np.asarray(weight, dtype=np.float32)
    bias = np.asarray(bias, dtype=np.float32)
    # wb[p, k*256+o] = W.T[k*128+p, o] = W[o, k*128+p]; wb[p, 512+t] = bias[t*128+p]
    wb = np.empty((P, WB_COLS), dtype=np.float32)
    wT = weight.T  # (I, O)
    for k in range(KT):
        wb[:, k * O:(k + 1) * O] = wT[k * P:(k + 1) * P, :]
    wb[:, KT * O:] = bias.reshape(OT, P).T
    wb = np.ascontiguousarray(wb)
    in_maps = []
    for c in range(N_CORES):
        in_maps.append({
            "xT": np.ascontiguousarray(x[c].T),
            "wb": wb,
        })
    return in_maps


def kernel(x, weight, bias):
    in_maps = _make_in_maps(x, weight, bias)
    res = _run(in_maps)
    out = np.empty((B, S, O), dtype=np.float32)
    for c in range(N_CORES):
        out[c] = res.results[c]["out"].T
    return out
